# revision 2
# baseline (speedup 1.0000x reference)
"""ConditioningMoEINR Trainium2 kernel — dense 8-core data-parallel.

Device kernel: feature-major activations [feat, pts]; weights as stationary
lhsT.  Sin range reduction: magic-round pair (period units) for wide-range
layers (pe/enc1/pol0), single ADD_RANGE_WRAP (radian units) for bounded
layers.  Routing in point-major [128, (chunk,subtile)*7]; single Exp table
switch at the end; fast DVE reciprocal for the top-2 renormalization.

Host orchestration: the first call goes through run_bass_kernel_spmd (which
under axon lowers via bass2jax/PJRT and compiles+caches the NEFF).  That
path rebuilds the jitted shard_map closure and re-ships every replicated
weight on every call, so subsequent calls use a cached jitted callable with
device-resident weights — only x (2 MB) moves per call.
"""

import sys

if "/opt/trn_rl_repo" not in sys.path:
    sys.path.insert(0, "/opt/trn_rl_repo")

import numpy as np

# ---- problem constants (hardcoded per contract) ----
N_TOTAL = 131072
N_CORES = 8
NPC = N_TOTAL // N_CORES          # 16384 points per core
CHUNK = 512                       # matmul moving-dim tile
NCHUNK = NPC // CHUNK             # 32
GROUP = 2                         # chunks per expert psum group
NGROUP = NCHUNK // GROUP          # 16
NSUB = CHUNK // 128               # 4 point-subtiles per chunk
NJ = NCHUNK * NSUB                # 128 point-major column groups
NUM_FREQ = 6
IN_F = 4
ENC = 128
POL = 64
EXP = 128
NE = 7
OMEGA = 30.0

PI = float(np.pi)
TWO_PI = float(2 * np.pi)
MAGIC = float(np.float32(1.5 * 2 ** 23))
S_PER = float(np.float32(OMEGA / (2 * np.pi)))   # radians->period units
BIG = 1.0e30

_CACHE = {}
_FAST = {}


def _build(npc):
    import concourse.bacc as bacc
    import concourse.mybir as mybir
    import concourse.tile as tile
    from contextlib import ExitStack

    DT = mybir.dt.float32
    AF = mybir.ActivationFunctionType
    ALU = mybir.AluOpType

    nchunk = npc // CHUNK
    ngroup = nchunk // GROUP
    nj = nchunk * NSUB

    nc = bacc.Bacc("TRN2", target_bir_lowering=False, debug=False)

    def din(name, shape):
        return nc.dram_tensor(name, list(shape), DT, kind="ExternalInput").ap()

    xT = din("xT", (IN_F, npc))
    pe_bs = din("pe_bs", (IN_F, 48))          # period-unit freq matrix
    pe_shift = din("pe_shift", (48, 1))       # 0 / 0.25 (cos rows)
    pe_bias = din("pe_bias", (48, 1))         # 0 / pi/2
    encW1a = din("encW1a", (48, ENC))         # sin/cos rows * 30/2pi
    encW1b = din("encW1b", (IN_F, ENC))       # x rows * 30/2pi
    encW2r = din("encW2r", (ENC, ENC))        # * 30
    polW0p = din("polW0p", (IN_F, POL))       # * 30/2pi
    polW1r = din("polW1r", (POL, POL))        # * 30
    polW2r = din("polW2r", (POL, POL))        # * 30
    polWl = din("polWl", (POL, NE))
    eW0a = din("eW0a", (NE, ENC, EXP))        # * 30
    eW0b = din("eW0b", (NE, POL, EXP))        # * 30
    eW1 = din("eW1", (NE, EXP, EXP))          # * 30
    eW2 = din("eW2", (NE, EXP, EXP))          # * 30
    eWo = din("eWo", (NE, EXP, NE))           # col-e padded Wo
    ident = din("ident", (128, 128))
    out_d = nc.dram_tensor("out", [npc], DT, kind="ExternalOutput").ap()

    with tile.TileContext(nc) as tc, ExitStack() as ctx:
        wpool = ctx.enter_context(tc.tile_pool(name="w", bufs=1))
        spool = ctx.enter_context(tc.tile_pool(name="s", bufs=3))
        s2pool = ctx.enter_context(tc.tile_pool(name="s2", bufs=4))
        gpool = ctx.enter_context(tc.tile_pool(name="g", bufs=3))
        rpool = ctx.enter_context(tc.tile_pool(name="r", bufs=1))
        ppool = ctx.enter_context(tc.tile_pool(name="ps", bufs=2, space="PSUM"))
        tpool = ctx.enter_context(tc.tile_pool(name="tr", bufs=2, space="PSUM"))

        # ---- resident weights ----
        _wn = [0]

        def wload(ap, shape):
            _wn[0] += 1
            t = wpool.tile(list(shape), DT, name=f"w{_wn[0]}", tag=f"w{_wn[0]}")
            nc.sync.dma_start(t[:], ap)
            return t

        w_pebs = wload(pe_bs[:], (IN_F, 48))
        w_peshift = wload(pe_shift[:], (48, 1))
        w_pebias = wload(pe_bias[:], (48, 1))
        w_enc1a = wload(encW1a[:], (48, ENC))
        w_enc1b = wload(encW1b[:], (IN_F, ENC))
        w_enc2 = wload(encW2r[:], (ENC, ENC))
        w_pol0 = wload(polW0p[:], (IN_F, POL))
        w_pol1 = wload(polW1r[:], (POL, POL))
        w_pol2 = wload(polW2r[:], (POL, POL))
        w_polWl = wload(polWl[:], (POL, NE))
        w_e0a = [wload(eW0a[e], (ENC, EXP)) for e in range(NE)]
        w_e0b = [wload(eW0b[e], (POL, EXP)) for e in range(NE)]
        w_e1 = [wload(eW1[e], (EXP, EXP)) for e in range(NE)]
        w_e2 = [wload(eW2[e], (EXP, EXP)) for e in range(NE)]
        w_eo = [wload(eWo[e], (EXP, NE)) for e in range(NE)]
        w_id = wload(ident[:], (128, 128))

        # point-major logits / preds for the whole core
        LT = rpool.tile([128, nj * NE], DT)
        PT = rpool.tile([128, nj * NE], DT)

        def magic_sin(dst, psum_ap, p, n, shift_ap, bias_ap):
            """dst = Sin(-2pi*((psum+shift+M)-M-psum) + bias); psum in period units."""
            tsh = spool.tile([p, n], DT, tag="tsh")
            if shift_ap is None:
                nc.vector.tensor_scalar_add(tsh[:p, :n], psum_ap, MAGIC)
            else:
                nc.vector.tensor_scalar(
                    tsh[:p, :n], psum_ap, shift_ap, MAGIC, op0=ALU.add, op1=ALU.add
                )
            u = spool.tile([p, n], DT, tag="u")
            nc.vector.scalar_tensor_tensor(
                u[:p, :n], tsh[:p, :n], MAGIC, psum_ap,
                op0=ALU.subtract, op1=ALU.subtract,
            )
            if bias_ap is None:
                nc.scalar.activation(dst, u[:p, :n], AF.Sin, scale=-TWO_PI)
            else:
                nc.scalar.activation(dst, u[:p, :n], AF.Sin, bias=bias_ap, scale=-TWO_PI)

        def wrap_sin(dst, psum_ap, p, n):
            """dst = Sin(wrap(psum)); psum in radians, |arg| < 3pi."""
            nc.vector.add_range_wrap(psum_ap, psum_ap, shift=0.0, bound=PI, period=TWO_PI)
            nc.scalar.activation(dst, psum_ap, AF.Sin)

        for g in range(ngroup):
            xg = spool.tile([IN_F, GROUP * CHUNK], DT, tag="xg")
            nc.sync.dma_start(xg[:], xT[:, g * GROUP * CHUNK:(g + 1) * GROUP * CHUNK])
            s2s = []
            sp2s = []
            for ci in range(GROUP):
                c = g * GROUP + ci
                xs = xg[:, ci * CHUNK:(ci + 1) * CHUNK]

                # --- positional encoding ---
                t48 = ppool.tile([128, CHUNK], DT, tag="trunk")
                nc.tensor.matmul(t48[0:48, :], w_pebs[:], xs, start=True, stop=True)
                pesin = spool.tile([48, CHUNK], DT, tag="pesin")
                magic_sin(pesin[:], t48[0:48, :], 48, CHUNK, w_peshift[:], w_pebias[:])

                # --- encoder ---
                h1 = ppool.tile([128, CHUNK], DT, tag="trunk")
                nc.tensor.matmul(h1[:], w_enc1a[:], pesin[:], start=True, stop=False)
                nc.tensor.matmul(h1[:], w_enc1b[:], xs, start=False, stop=True)
                s1 = spool.tile([ENC, CHUNK], DT, tag="s1")
                magic_sin(s1[:], h1[:], ENC, CHUNK, None, None)

                h2 = ppool.tile([128, CHUNK], DT, tag="trunk")
                nc.tensor.matmul(h2[:], w_enc2[:], s1[:], start=True, stop=True)
                s2 = s2pool.tile([ENC, CHUNK], DT, tag="s2")
                wrap_sin(s2[:], h2[:], ENC, CHUNK)
                s2s.append(s2)

                # --- policy ---
                p0 = ppool.tile([128, CHUNK], DT, tag="trunk")
                nc.tensor.matmul(p0[0:POL, :], w_pol0[:], xs, start=True, stop=True)
                sp0 = spool.tile([POL, CHUNK], DT, tag="sp0")
                magic_sin(sp0[:], p0[0:POL, :], POL, CHUNK, None, None)

                p1 = ppool.tile([128, CHUNK], DT, tag="trunk")
                nc.tensor.matmul(p1[0:POL, :], w_pol1[:], sp0[:], start=True, stop=True)
                sp1 = spool.tile([POL, CHUNK], DT, tag="sp1")
                wrap_sin(sp1[:], p1[0:POL, :], POL, CHUNK)

                p2 = ppool.tile([128, CHUNK], DT, tag="trunk")
                nc.tensor.matmul(p2[0:POL, :], w_pol2[:], sp1[:], start=True, stop=True)
                sp2 = s2pool.tile([POL, CHUNK], DT, tag="sp2")
                wrap_sin(sp2[:], p2[0:POL, :], POL, CHUNK)
                sp2s.append(sp2)

                # --- logits, point-major [128, 28] ---
                lt = ppool.tile([128, NSUB * NE], DT, tag="trunk")
                for s in range(NSUB):
                    nc.tensor.matmul(
                        lt[:, s * NE:(s + 1) * NE],
                        sp2[:, s * 128:(s + 1) * 128],
                        w_polWl[:],
                        start=True, stop=True,
                    )
                nc.vector.tensor_copy(LT[:, c * NSUB * NE:(c + 1) * NSUB * NE], lt[:, 0:NSUB * NE])

            # --- experts (grouped over GROUP chunks) ---
            p7s = [tpool.tile([NE, CHUNK], DT, tag="p7", name=f"p7_{g}_{i}") for i in range(GROUP)]
            for e in range(NE):
                x0 = ppool.tile([128, GROUP * CHUNK], DT, tag="exp")
                for ci in range(GROUP):
                    sl = x0[:, ci * CHUNK:(ci + 1) * CHUNK]
                    nc.tensor.matmul(sl, w_e0a[e][:], s2s[ci][:], start=True, stop=False)
                    nc.tensor.matmul(sl, w_e0b[e][:], sp2s[ci][:], start=False, stop=True)
                g0 = gpool.tile([EXP, GROUP * CHUNK], DT, tag="g0")
                wrap_sin(g0[:], x0[:], EXP, GROUP * CHUNK)

                x1 = ppool.tile([128, GROUP * CHUNK], DT, tag="exp")
                for ci in range(GROUP):
                    nc.tensor.matmul(
                        x1[:, ci * CHUNK:(ci + 1) * CHUNK], w_e1[e][:],
                        g0[:, ci * CHUNK:(ci + 1) * CHUNK], start=True, stop=True,
                    )
                g1 = gpool.tile([EXP, GROUP * CHUNK], DT, tag="g1")
                wrap_sin(g1[:], x1[:], EXP, GROUP * CHUNK)

                x2 = ppool.tile([128, GROUP * CHUNK], DT, tag="exp")
                for ci in range(GROUP):
                    nc.tensor.matmul(
                        x2[:, ci * CHUNK:(ci + 1) * CHUNK], w_e2[e][:],
                        g1[:, ci * CHUNK:(ci + 1) * CHUNK], start=True, stop=True,
                    )
                g2 = gpool.tile([EXP, GROUP * CHUNK], DT, tag="g2")
                wrap_sin(g2[:], x2[:], EXP, GROUP * CHUNK)

                for ci in range(GROUP):
                    nc.tensor.matmul(
                        p7s[ci][0:NE, :], w_eo[e][:],
                        g2[:, ci * CHUNK:(ci + 1) * CHUNK],
                        start=(e == 0), stop=(e == NE - 1),
                    )

            # preds -> point-major PT via PE transpose
            for ci in range(GROUP):
                c = g * GROUP + ci
                tmp7 = spool.tile([NE, CHUNK], DT, tag="tmp7")
                nc.vector.tensor_copy(tmp7[:], p7s[ci][0:NE, :])
                tp = ppool.tile([128, NSUB * NE], DT, tag="trunk")
                for s in range(NSUB):
                    nc.tensor.transpose(
                        tp[:, s * NE:(s + 1) * NE],
                        tmp7[:, s * 128:(s + 1) * 128],
                        w_id[0:NE, 0:NE],
                    )
                nc.vector.tensor_copy(PT[:, c * NSUB * NE:(c + 1) * NSUB * NE], tp[:, 0:NSUB * NE])

        # ================= phase B: routing + combine =================
        LT3 = LT[:].rearrange("p (j e) -> p j e", e=NE)

        def etree(op, src3, width_tag):
            """pairwise tree over the 7-expert innermost dim -> [128, nj]."""
            m4 = rpool.tile([128, nj * 4], DT, tag=f"{width_tag}4")
            m43 = m4[:].rearrange("p (j e) -> p j e", e=4)
            nc.vector.tensor_tensor(m43, src3[:, :, 0:4], src3[:, :, 3:7], op)
            m2 = rpool.tile([128, nj * 2], DT, tag=f"{width_tag}2")
            m23 = m2[:].rearrange("p (j e) -> p j e", e=2)
            nc.vector.tensor_tensor(m23, m43[:, :, 0:2], m43[:, :, 2:4], op)
            m1 = rpool.tile([128, nj], DT, tag=f"{width_tag}1")
            m13 = m1[:].rearrange("p (j e) -> p j e", e=1)
            nc.vector.tensor_tensor(m13, m23[:, :, 0:1], m23[:, :, 1:2], op)
            return m1

        def erep(m1, tag):
            """broadcast [128, nj] -> [128, nj*7] along innermost expert dim."""
            r = rpool.tile([128, nj * NE], DT, tag=tag)
            r3 = r[:].rearrange("p (j e) -> p j e", e=NE)
            m13 = m1[:].rearrange("p (j e) -> p j e", e=1)
            nc.vector.tensor_copy(r3[:, :, 0:1], m13)
            nc.vector.tensor_copy(r3[:, :, 1:2], r3[:, :, 0:1])
            nc.vector.tensor_copy(r3[:, :, 2:4], r3[:, :, 0:2])
            nc.vector.tensor_copy(r3[:, :, 4:7], r3[:, :, 1:4])
            return r, r3

        mx1 = etree(ALU.max, LT3, "mxa")
        rep1, rep13 = erep(mx1, "rep1")
        # masked' = BIG*ge1 - LT  (negated; use min-tree then negate)
        ge1 = rpool.tile([128, nj * NE], DT, tag="ge1")
        nc.vector.tensor_tensor(ge1[:], LT[:], rep1[:], ALU.is_ge)
        maskd = rpool.tile([128, nj * NE], DT, tag="maskd")
        nc.vector.scalar_tensor_tensor(
            maskd[:], ge1[:], BIG, LT[:], op0=ALU.mult, op1=ALU.subtract
        )
        mn = etree(ALU.min, maskd[:].rearrange("p (j e) -> p j e", e=NE), "mna")
        mx2 = rpool.tile([128, nj], DT, tag="mx2")
        nc.vector.tensor_scalar_mul(mx2[:], mn[:], -1.0)
        rep2, _ = erep(mx2, "rep2")
        keep = rpool.tile([128, nj * NE], DT, tag="keep")
        nc.vector.tensor_tensor(keep[:], LT[:], rep2[:], ALU.is_ge)

        ex = rpool.tile([128, nj * NE], DT, tag="ex")
        nc.scalar.activation(ex[:], LT[:], AF.Exp)
        ew = rpool.tile([128, nj * NE], DT, tag="ew")
        nc.vector.tensor_tensor(ew[:], ex[:], keep[:], ALU.mult)
        wp = rpool.tile([128, nj * NE], DT, tag="wp")
        nc.vector.tensor_tensor(wp[:], ew[:], PT[:], ALU.mult)

        den = rpool.tile([128, nj], DT, tag="den")
        nc.vector.tensor_reduce(
            den[:], ew[:].rearrange("p (j e) -> p j e", e=NE),
            mybir.AxisListType.X, ALU.add,
        )
        num = rpool.tile([128, nj], DT, tag="num")
        nc.vector.tensor_reduce(
            num[:], wp[:].rearrange("p (j e) -> p j e", e=NE),
            mybir.AxisListType.X, ALU.add,
        )
        rec = rpool.tile([128, nj], DT, tag="rec")
        scratch = rpool.tile([128, nj], DT, tag="recs")
        nc.vector.reciprocal_approx_accurate(rec[:], den[:], scratch[:])
        outv = rpool.tile([128, nj], DT, tag="outv")
        nc.vector.tensor_tensor(outv[:], num[:], rec[:], ALU.mult)

        # transpose [128 q, nj] -> [nj, 128 q] and store
        for b in range(nj // 128):
            tp = ppool.tile([128, 128], DT, tag="exp")
            nc.tensor.transpose(tp[:, 0:128], outv[:, b * 128:(b + 1) * 128], w_id[:])
            osb = rpool.tile([128, 128], DT, tag="osb")
            nc.vector.tensor_copy(osb[:], tp[:, 0:128])
            nc.sync.dma_start(
                out_d.rearrange("(j q) -> j q", q=128)[b * 128:(b + 1) * 128, :],
                osb[:],
            )

    nc.compile()
    return nc


def _prep_weights(inputs):
    """Weight-derived device inputs (shared by all cores)."""
    f32 = np.float32
    S30 = f32(OMEGA)
    SP = f32(OMEGA / (2 * np.pi))

    # pe freq matrix in period units: col j=i*6+k (sin), 24+j (cos) = 2^(k-1)
    pe_bs = np.zeros((IN_F, 48), f32)
    for i in range(IN_F):
        for k in range(NUM_FREQ):
            pe_bs[i, i * NUM_FREQ + k] = 2.0 ** (k - 1)
            pe_bs[i, 24 + i * NUM_FREQ + k] = 2.0 ** (k - 1)
    pe_shift = np.zeros((48, 1), f32)
    pe_shift[24:48] = 0.25
    pe_bias = (pe_shift * f32(2 * np.pi)).astype(f32)

    # enc_W1 rows permuted to [sin/cos(48); x(4)], scaled to period units
    encW1 = inputs["enc_W1"].astype(f32)
    encW1p = np.concatenate([encW1[4:52], encW1[0:4]], axis=0) * SP

    d = {
        "pe_bs": pe_bs,
        "pe_shift": pe_shift,
        "pe_bias": pe_bias,
        "encW1a": np.ascontiguousarray(encW1p[0:48]).astype(f32),
        "encW1b": np.ascontiguousarray(encW1p[48:52]).astype(f32),
        "encW2r": (inputs["enc_W2"].astype(f32) * S30),
        "polW0p": (inputs["pol_W0"].astype(f32)[0:IN_F] * SP),
        "polW1r": (inputs["pol_W1"].astype(f32) * S30),
        "polW2r": (inputs["pol_W2"].astype(f32) * S30),
        "polWl": inputs["pol_Wl"].astype(f32),
        "eW0a": np.ascontiguousarray(inputs["exp_W0"].astype(f32)[:, 0:ENC, :] * S30),
        "eW0b": np.ascontiguousarray(inputs["exp_W0"].astype(f32)[:, ENC:ENC + POL, :] * S30),
        "eW1": inputs["exp_W1"].astype(f32) * S30,
        "eW2": inputs["exp_W2"].astype(f32) * S30,
        "ident": np.eye(128, dtype=f32),
    }
    eWo = np.zeros((NE, EXP, NE), f32)
    for e in range(NE):
        eWo[e, :, e] = inputs["exp_Wo"][e, :, 0]
    d["eWo"] = eWo

    # biases are structurally zero in this model; the kernel folds none.
    for b in ["enc_b1", "enc_b2", "pol_b0", "pol_b1", "pol_b2", "pol_bl",
              "exp_b0", "exp_b1", "exp_b2", "exp_bo"]:
        assert not np.any(inputs[b]), f"nonzero bias {b} unsupported"

    return d


def _x_concat(x):
    """x [N,4] -> concatenated per-core xT blocks, shape (8*IN_F, NPC)."""
    f32 = np.float32
    return np.ascontiguousarray(
        x.astype(f32, copy=False).reshape(N_CORES, NPC, IN_F).transpose(0, 2, 1)
    ).reshape(N_CORES * IN_F, NPC)


_WKEYS = ["enc_W1", "enc_W2", "pol_W0", "pol_W1", "pol_W2", "pol_Wl",
          "exp_W0", "exp_W1", "exp_W2", "exp_Wo"]


def _weight_sig(inputs):
    return tuple(
        (k, id(inputs[k]), inputs[k].__array_interface__["data"][0])
        for k in _WKEYS
    )


def _make_fast(nc, wmap):
    """Cached jitted shard_map callable mirroring run_bass_via_pjrt exactly,
    with weights device-resident."""
    import jax
    from jax.sharding import Mesh, PartitionSpec, NamedSharding
    from jax.experimental.shard_map import shard_map
    from concourse import mybir
    from concourse.bass2jax import (
        _bass_exec_p,
        install_neuronx_cc_hook,
        partition_id_tensor,
    )

    install_neuronx_cc_hook()
    partition_name = nc.partition_id_tensor.name if nc.partition_id_tensor else None

    in_names, out_names, out_avals, zero_shapes = [], [], [], []
    for alloc in nc.m.functions[0].allocations:
        if not isinstance(alloc, mybir.MemoryLocationSet):
            continue
        name = alloc.memorylocations[0].name
        if alloc.kind == "ExternalInput":
            if name != partition_name:
                in_names.append(name)
        elif alloc.kind == "ExternalOutput":
            shape = tuple(alloc.tensor_shape)
            dtype = mybir.dt.np(alloc.dtype)
            out_avals.append(jax.core.ShapedArray(shape, dtype))
            zero_shapes.append((shape, dtype))
            out_names.append(name)
    n_params = len(in_names)
    n_outs = len(out_avals)
    in_names_full = in_names + out_names
    if partition_name is not None:
        in_names_full.append(partition_name)

    def _body(*args):
        operands = list(args)
        if partition_name is not None:
            operands.append(partition_id_tensor())
        outs = _bass_exec_p.bind(
            *operands,
            out_avals=tuple(out_avals),
            in_names=tuple(in_names_full),
            out_names=tuple(out_names),
            lowering_input_output_aliases=(),
            sim_require_finite=True,
            sim_require_nnan=True,
            nc=nc,
        )
        return tuple(outs)

    devices = jax.devices()[:N_CORES]
    mesh = Mesh(np.asarray(devices), ("core",))
    in_specs = (PartitionSpec("core"),) * (n_params + n_outs)
    out_specs = (PartitionSpec("core"),) * len(out_names)
    donate = tuple(range(n_params, n_params + n_outs))
    sharded = jax.jit(
        shard_map(_body, mesh=mesh, in_specs=in_specs, out_specs=out_specs,
                  check_rep=False),
        donate_argnums=donate,
        keep_unused=True,
    )
    shard = NamedSharding(mesh, PartitionSpec("core"))

    x_idx = in_names.index("xT")

    def put_weights(wmap):
        dev = {}
        for name in in_names:
            if name == "xT":
                continue
            w = wmap[name]
            rep = np.broadcast_to(w[None], (N_CORES, *w.shape)).reshape(
                N_CORES * w.shape[0], *w.shape[1:]
            )
            dev[name] = jax.device_put(np.ascontiguousarray(rep), shard)
        return dev

    state = {
        "sharded": sharded,
        "in_names": in_names,
        "out_names": out_names,
        "zero_shapes": zero_shapes,
        "x_idx": x_idx,
        "put_weights": put_weights,
        "dev_weights": put_weights(wmap),
    }
    return state


def _fast_call(state, x):
    args = []
    for name in state["in_names"]:
        if name == "xT":
            args.append(_x_concat(x))
        else:
            args.append(state["dev_weights"][name])
    zeros = [np.zeros((N_CORES * s[0], *s[1:]), dt)
             for (s, dt) in state["zero_shapes"]]
    out_arrs = state["sharded"](*args, *zeros)
    out = np.asarray(out_arrs[state["out_names"].index("out")])
    return out.reshape(N_TOTAL, 1).astype(np.float32)


def kernel(**inputs):
    from concourse.bass_utils import run_bass_kernel_spmd

    npc = NPC
    if npc not in _CACHE:
        _CACHE[npc] = _build(npc)
    nc = _CACHE[npc]

    sig = _weight_sig(inputs)
    st = _FAST.get(npc)
    if st is not None and st["sig"] == sig:
        return _fast_call(st["state"], inputs["x"])

    # First call (or weights changed): canonical run_bass_kernel_spmd path.
    wmap = _prep_weights(inputs)
    x = inputs["x"].astype(np.float32, copy=False)
    in_maps = []
    for core in range(N_CORES):
        m = dict(wmap)
        m["xT"] = np.ascontiguousarray(x[core * npc:(core + 1) * npc].T)
        in_maps.append(m)
    res = run_bass_kernel_spmd(nc, in_maps, list(range(N_CORES)))
    out = np.concatenate([res.results[c]["out"] for c in range(N_CORES)])
    out = out.reshape(N_TOTAL, 1).astype(np.float32)

    # Build/refresh the fast path for subsequent calls; validate it once.
    try:
        if st is None:
            st = {"state": _make_fast(nc, wmap), "sig": sig}
        else:
            st["state"]["dev_weights"] = st["state"]["put_weights"](wmap)
            st["sig"] = sig
        fast_out = _fast_call(st["state"], inputs["x"])
        if np.allclose(fast_out, out, rtol=1e-5, atol=1e-6):
            _FAST[npc] = st
        else:
            _FAST.pop(npc, None)
    except Exception:
        _FAST.pop(npc, None)

    return out


# revision 11
# speedup vs baseline: 1.0066x; 1.0066x over previous
"""ConditioningMoEINR Trainium2 kernel — dense 8-core data-parallel.

Device kernel: feature-major activations [feat, pts]; weights as stationary
lhsT.  Sin range reduction: magic-round pair (period units) for wide-range
layers (pe/enc1/pol0), single ADD_RANGE_WRAP (radian units) for bounded
layers.  Routing in point-major [128, (chunk,subtile)*7]; single Exp table
switch at the end; fast DVE reciprocal for the top-2 renormalization.

Host orchestration: the first call goes through run_bass_kernel_spmd (which
under axon lowers via bass2jax/PJRT and compiles+caches the NEFF).  That
path rebuilds the jitted shard_map closure and re-ships every replicated
weight on every call, so subsequent calls use a cached jitted callable with
device-resident weights — only x (2 MB) moves per call.
"""

import sys

if "/opt/trn_rl_repo" not in sys.path:
    sys.path.insert(0, "/opt/trn_rl_repo")

import numpy as np

# ---- problem constants (hardcoded per contract) ----
N_TOTAL = 131072
N_CORES = 8
NPC = N_TOTAL // N_CORES          # 16384 points per core
CHUNK = 512                       # matmul moving-dim tile
NCHUNK = NPC // CHUNK             # 32
GROUP = 2                         # chunks per expert psum group
NGROUP = NCHUNK // GROUP          # 16
NSUB = CHUNK // 128               # 4 point-subtiles per chunk
NJ = NCHUNK * NSUB                # 128 point-major column groups
NUM_FREQ = 6
IN_F = 4
ENC = 128
POL = 64
EXP = 128
NE = 7
OMEGA = 30.0

PI = float(np.pi)
TWO_PI = float(2 * np.pi)
MAGIC = float(np.float32(1.5 * 2 ** 23))
S_PER = float(np.float32(OMEGA / (2 * np.pi)))   # radians->period units
BIG = 1.0e30

_CACHE = {}
_FAST = {}


def _build(npc):
    import concourse.bacc as bacc
    import concourse.mybir as mybir
    import concourse.tile as tile
    from contextlib import ExitStack

    DT = mybir.dt.float32
    DTB = mybir.dt.bfloat16
    AF = mybir.ActivationFunctionType
    ALU = mybir.AluOpType

    nchunk = npc // CHUNK
    ngroup = nchunk // GROUP
    nj = nchunk * NSUB

    nc = bacc.Bacc("TRN2", target_bir_lowering=False, debug=False)

    def din(name, shape, dt=DT):
        return nc.dram_tensor(name, list(shape), dt, kind="ExternalInput").ap()

    xT = din("xT", (IN_F, npc))
    pe_bs = din("pe_bs", (IN_F, 48))          # period-unit freq matrix
    pe_shift = din("pe_shift", (48, 1))       # 0 / 0.25 (cos rows)
    pe_bias = din("pe_bias", (48, 1))         # 0 / pi/2
    encW1a = din("encW1a", (48, ENC))         # sin/cos rows * 30/2pi
    encW1b = din("encW1b", (IN_F, ENC))       # x rows * 30/2pi
    encW2r = din("encW2r", (ENC, ENC), DTB)   # * 30
    polW0p = din("polW0p", (IN_F, POL))       # * 30/2pi
    polW1r = din("polW1r", (POL, POL))        # * 30
    polW2r = din("polW2r", (POL, POL))        # * 30
    polWl = din("polWl", (POL, NE))
    eW0a = din("eW0a", (NE, ENC, EXP), DTB)   # * 30
    eW0b = din("eW0b", (NE, POL, EXP), DTB)   # * 30
    eW1 = din("eW1", (NE, EXP, EXP), DTB)     # * 30
    eW2 = din("eW2", (NE, EXP, EXP), DTB)     # * 30
    eWo = din("eWo", (NE, EXP, NE), DTB)      # col-e padded Wo
    ident = din("ident", (128, 128))
    out_d = nc.dram_tensor("out", [npc], DT, kind="ExternalOutput").ap()

    with tile.TileContext(nc) as tc, ExitStack() as ctx:
        wpool = ctx.enter_context(tc.tile_pool(name="w", bufs=1))
        spool = ctx.enter_context(tc.tile_pool(name="s", bufs=3))
        s2pool = ctx.enter_context(tc.tile_pool(name="s2", bufs=4))
        gpool = ctx.enter_context(tc.tile_pool(name="g", bufs=3))
        rpool = ctx.enter_context(tc.tile_pool(name="r", bufs=1))
        ppool = ctx.enter_context(tc.tile_pool(name="ps", bufs=2, space="PSUM"))
        tpool = ctx.enter_context(tc.tile_pool(name="tr", bufs=2, space="PSUM"))

        # ---- resident weights ----
        _wn = [0]

        def wload(ap, shape, dt=DT):
            _wn[0] += 1
            t = wpool.tile(list(shape), dt, name=f"w{_wn[0]}", tag=f"w{_wn[0]}")
            nc.sync.dma_start(t[:], ap)
            return t

        w_pebs = wload(pe_bs[:], (IN_F, 48))
        w_peshift = wload(pe_shift[:], (48, 1))
        w_pebias = wload(pe_bias[:], (48, 1))
        w_enc1a = wload(encW1a[:], (48, ENC))
        w_enc1b = wload(encW1b[:], (IN_F, ENC))
        w_enc2 = wload(encW2r[:], (ENC, ENC), DTB)
        w_pol0 = wload(polW0p[:], (IN_F, POL))
        w_pol1 = wload(polW1r[:], (POL, POL))
        w_pol2 = wload(polW2r[:], (POL, POL))
        w_polWl = wload(polWl[:], (POL, NE))
        w_e0a = [wload(eW0a[e], (ENC, EXP), DTB) for e in range(NE)]
        w_e0b = [wload(eW0b[e], (POL, EXP), DTB) for e in range(NE)]
        w_e1 = [wload(eW1[e], (EXP, EXP), DTB) for e in range(NE)]
        w_e2 = [wload(eW2[e], (EXP, EXP), DTB) for e in range(NE)]
        w_eo = [wload(eWo[e], (EXP, NE), DTB) for e in range(NE)]
        w_id = wload(ident[:], (128, 128))

        # point-major logits / preds for the whole core
        LT = rpool.tile([128, nj * NE], DT)
        PT = rpool.tile([128, nj * NE], DT)

        def magic_sin(dst, psum_ap, p, n, shift_ap, bias_ap):
            """dst = Sin(-2pi*((psum+shift+M)-M-psum) + bias); psum in period units."""
            tsh = spool.tile([p, n], DT, tag="tsh")
            if shift_ap is None:
                nc.vector.tensor_scalar_add(tsh[:p, :n], psum_ap, MAGIC)
            else:
                nc.vector.tensor_scalar(
                    tsh[:p, :n], psum_ap, shift_ap, MAGIC, op0=ALU.add, op1=ALU.add
                )
            u = spool.tile([p, n], DT, tag="u")
            nc.vector.scalar_tensor_tensor(
                u[:p, :n], tsh[:p, :n], MAGIC, psum_ap,
                op0=ALU.subtract, op1=ALU.subtract,
            )
            if bias_ap is None:
                nc.scalar.activation(dst, u[:p, :n], AF.Sin, scale=-TWO_PI)
            else:
                nc.scalar.activation(dst, u[:p, :n], AF.Sin, bias=bias_ap, scale=-TWO_PI)

        def wrap_sin(dst, psum_ap, p, n):
            """dst = Sin(wrap(psum)); psum in radians, |arg| < 3pi."""
            nc.vector.add_range_wrap(psum_ap, psum_ap, shift=0.0, bound=PI, period=TWO_PI)
            nc.scalar.activation(dst, psum_ap, AF.Sin)

        for g in range(ngroup):
            xg = spool.tile([IN_F, GROUP * CHUNK], DT, tag="xg")
            nc.sync.dma_start(xg[:], xT[:, g * GROUP * CHUNK:(g + 1) * GROUP * CHUNK])
            s2s = []
            sp2s = []
            for ci in range(GROUP):
                c = g * GROUP + ci
                xs = xg[:, ci * CHUNK:(ci + 1) * CHUNK]

                # --- positional encoding ---
                t48 = ppool.tile([128, CHUNK], DT, tag="trunk")
                nc.tensor.matmul(t48[0:48, :], w_pebs[:], xs, start=True, stop=True)
                pesin = spool.tile([48, CHUNK], DT, tag="pesin")
                magic_sin(pesin[:], t48[0:48, :], 48, CHUNK, w_peshift[:], w_pebias[:])

                # --- encoder ---
                h1 = ppool.tile([128, CHUNK], DT, tag="trunk")
                nc.tensor.matmul(h1[:], w_enc1a[:], pesin[:], start=True, stop=False)
                nc.tensor.matmul(h1[:], w_enc1b[:], xs, start=False, stop=True)
                s1 = spool.tile([ENC, CHUNK], DTB, tag="s1")
                magic_sin(s1[:], h1[:], ENC, CHUNK, None, None)

                h2 = ppool.tile([128, CHUNK], DT, tag="trunk")
                nc.tensor.matmul(h2[:], w_enc2[:], s1[:], start=True, stop=True)
                s2 = s2pool.tile([ENC, CHUNK], DTB, tag="s2")
                wrap_sin(s2[:], h2[:], ENC, CHUNK)
                s2s.append(s2)

                # --- policy ---
                p0 = ppool.tile([128, CHUNK], DT, tag="trunk")
                nc.tensor.matmul(p0[0:POL, :], w_pol0[:], xs, start=True, stop=True)
                sp0 = spool.tile([POL, CHUNK], DT, tag="sp0")
                magic_sin(sp0[:], p0[0:POL, :], POL, CHUNK, None, None)

                p1 = ppool.tile([128, CHUNK], DT, tag="trunk")
                nc.tensor.matmul(p1[0:POL, :], w_pol1[:], sp0[:], start=True, stop=True)
                sp1 = spool.tile([POL, CHUNK], DT, tag="sp1")
                wrap_sin(sp1[:], p1[0:POL, :], POL, CHUNK)

                p2 = ppool.tile([128, CHUNK], DT, tag="trunk")
                nc.tensor.matmul(p2[0:POL, :], w_pol2[:], sp1[:], start=True, stop=True)
                sp2 = s2pool.tile([POL, CHUNK], DT, tag="sp2")
                wrap_sin(sp2[:], p2[0:POL, :], POL, CHUNK)
                sp2b = s2pool.tile([POL, CHUNK], DTB, tag="sp2b")
                nc.vector.tensor_copy(sp2b[:], sp2[:])
                sp2s.append(sp2b)

                # --- logits, point-major [128, 28] ---
                lt = ppool.tile([128, NSUB * NE], DT, tag="trunk")
                for s in range(NSUB):
                    nc.tensor.matmul(
                        lt[:, s * NE:(s + 1) * NE],
                        sp2[:, s * 128:(s + 1) * 128],
                        w_polWl[:],
                        start=True, stop=True,
                    )
                nc.vector.tensor_copy(LT[:, c * NSUB * NE:(c + 1) * NSUB * NE], lt[:, 0:NSUB * NE])

            # --- experts (grouped over GROUP chunks) ---
            p7s = [tpool.tile([NE, CHUNK], DT, tag="p7", name=f"p7_{g}_{i}") for i in range(GROUP)]
            for e in range(NE):
                x0 = ppool.tile([128, GROUP * CHUNK], DT, tag="exp")
                for ci in range(GROUP):
                    sl = x0[:, ci * CHUNK:(ci + 1) * CHUNK]
                    nc.tensor.matmul(sl, w_e0a[e][:], s2s[ci][:], start=True, stop=False)
                    nc.tensor.matmul(sl, w_e0b[e][:], sp2s[ci][:], start=False, stop=True)
                g0 = gpool.tile([EXP, GROUP * CHUNK], DTB, tag="g0")
                wrap_sin(g0[:], x0[:], EXP, GROUP * CHUNK)

                x1 = ppool.tile([128, GROUP * CHUNK], DT, tag="exp")
                for ci in range(GROUP):
                    nc.tensor.matmul(
                        x1[:, ci * CHUNK:(ci + 1) * CHUNK], w_e1[e][:],
                        g0[:, ci * CHUNK:(ci + 1) * CHUNK], start=True, stop=True,
                    )
                g1 = gpool.tile([EXP, GROUP * CHUNK], DTB, tag="g1")
                wrap_sin(g1[:], x1[:], EXP, GROUP * CHUNK)

                x2 = ppool.tile([128, GROUP * CHUNK], DT, tag="exp")
                for ci in range(GROUP):
                    nc.tensor.matmul(
                        x2[:, ci * CHUNK:(ci + 1) * CHUNK], w_e2[e][:],
                        g1[:, ci * CHUNK:(ci + 1) * CHUNK], start=True, stop=True,
                    )
                g2 = gpool.tile([EXP, GROUP * CHUNK], DTB, tag="g2")
                wrap_sin(g2[:], x2[:], EXP, GROUP * CHUNK)

                for ci in range(GROUP):
                    nc.tensor.matmul(
                        p7s[ci][0:NE, :], w_eo[e][:],
                        g2[:, ci * CHUNK:(ci + 1) * CHUNK],
                        start=(e == 0), stop=(e == NE - 1),
                    )

            # preds -> point-major PT via PE transpose
            for ci in range(GROUP):
                c = g * GROUP + ci
                tmp7 = spool.tile([NE, CHUNK], DT, tag="tmp7")
                nc.vector.tensor_copy(tmp7[:], p7s[ci][0:NE, :])
                tp = ppool.tile([128, NSUB * NE], DT, tag="trunk")
                for s in range(NSUB):
                    nc.tensor.transpose(
                        tp[:, s * NE:(s + 1) * NE],
                        tmp7[:, s * 128:(s + 1) * 128],
                        w_id[0:NE, 0:NE],
                    )
                nc.vector.tensor_copy(PT[:, c * NSUB * NE:(c + 1) * NSUB * NE], tp[:, 0:NSUB * NE])

        # ================= phase B: routing + combine =================
        LT3 = LT[:].rearrange("p (j e) -> p j e", e=NE)

        def etree(op, src3, width_tag):
            """pairwise tree over the 7-expert innermost dim -> [128, nj]."""
            m4 = rpool.tile([128, nj * 4], DT, tag=f"{width_tag}4")
            m43 = m4[:].rearrange("p (j e) -> p j e", e=4)
            nc.vector.tensor_tensor(m43, src3[:, :, 0:4], src3[:, :, 3:7], op)
            m2 = rpool.tile([128, nj * 2], DT, tag=f"{width_tag}2")
            m23 = m2[:].rearrange("p (j e) -> p j e", e=2)
            nc.vector.tensor_tensor(m23, m43[:, :, 0:2], m43[:, :, 2:4], op)
            m1 = rpool.tile([128, nj], DT, tag=f"{width_tag}1")
            m13 = m1[:].rearrange("p (j e) -> p j e", e=1)
            nc.vector.tensor_tensor(m13, m23[:, :, 0:1], m23[:, :, 1:2], op)
            return m1

        def erep(m1, tag):
            """broadcast [128, nj] -> [128, nj*7] along innermost expert dim."""
            r = rpool.tile([128, nj * NE], DT, tag=tag)
            r3 = r[:].rearrange("p (j e) -> p j e", e=NE)
            m13 = m1[:].rearrange("p (j e) -> p j e", e=1)
            nc.vector.tensor_copy(r3[:, :, 0:1], m13)
            nc.vector.tensor_copy(r3[:, :, 1:2], r3[:, :, 0:1])
            nc.vector.tensor_copy(r3[:, :, 2:4], r3[:, :, 0:2])
            nc.vector.tensor_copy(r3[:, :, 4:7], r3[:, :, 1:4])
            return r, r3

        mx1 = etree(ALU.max, LT3, "mxa")
        rep1, rep13 = erep(mx1, "rep1")
        # masked' = BIG*ge1 - LT  (negated; use min-tree then negate)
        ge1 = rpool.tile([128, nj * NE], DT, tag="ge1")
        nc.vector.tensor_tensor(ge1[:], LT[:], rep1[:], ALU.is_ge)
        maskd = rpool.tile([128, nj * NE], DT, tag="maskd")
        nc.vector.scalar_tensor_tensor(
            maskd[:], ge1[:], BIG, LT[:], op0=ALU.mult, op1=ALU.subtract
        )
        mn = etree(ALU.min, maskd[:].rearrange("p (j e) -> p j e", e=NE), "mna")
        mx2 = rpool.tile([128, nj], DT, tag="mx2")
        nc.vector.tensor_scalar_mul(mx2[:], mn[:], -1.0)
        rep2, _ = erep(mx2, "rep2")
        keep = rpool.tile([128, nj * NE], DT, tag="keep")
        nc.vector.tensor_tensor(keep[:], LT[:], rep2[:], ALU.is_ge)

        ex = rpool.tile([128, nj * NE], DT, tag="ex")
        nc.scalar.activation(ex[:], LT[:], AF.Exp)
        ew = rpool.tile([128, nj * NE], DT, tag="ew")
        nc.vector.tensor_tensor(ew[:], ex[:], keep[:], ALU.mult)
        wp = rpool.tile([128, nj * NE], DT, tag="wp")
        nc.vector.tensor_tensor(wp[:], ew[:], PT[:], ALU.mult)

        den = rpool.tile([128, nj], DT, tag="den")
        nc.vector.tensor_reduce(
            den[:], ew[:].rearrange("p (j e) -> p j e", e=NE),
            mybir.AxisListType.X, ALU.add,
        )
        num = rpool.tile([128, nj], DT, tag="num")
        nc.vector.tensor_reduce(
            num[:], wp[:].rearrange("p (j e) -> p j e", e=NE),
            mybir.AxisListType.X, ALU.add,
        )
        rec = rpool.tile([128, nj], DT, tag="rec")
        scratch = rpool.tile([128, nj], DT, tag="recs")
        nc.vector.reciprocal_approx_accurate(rec[:], den[:], scratch[:])
        outv = rpool.tile([128, nj], DT, tag="outv")
        nc.vector.tensor_tensor(outv[:], num[:], rec[:], ALU.mult)

        # transpose [128 q, nj] -> [nj, 128 q] and store
        for b in range(nj // 128):
            tp = ppool.tile([128, 128], DT, tag="exp")
            nc.tensor.transpose(tp[:, 0:128], outv[:, b * 128:(b + 1) * 128], w_id[:])
            osb = rpool.tile([128, 128], DT, tag="osb")
            nc.vector.tensor_copy(osb[:], tp[:, 0:128])
            nc.sync.dma_start(
                out_d.rearrange("(j q) -> j q", q=128)[b * 128:(b + 1) * 128, :],
                osb[:],
            )

    nc.compile()
    return nc


def _prep_weights(inputs):
    """Weight-derived device inputs (shared by all cores)."""
    import ml_dtypes

    f32 = np.float32
    bf16 = ml_dtypes.bfloat16
    S30 = f32(OMEGA)
    SP = f32(OMEGA / (2 * np.pi))

    # pe freq matrix in period units: col j=i*6+k (sin), 24+j (cos) = 2^(k-1)
    pe_bs = np.zeros((IN_F, 48), f32)
    for i in range(IN_F):
        for k in range(NUM_FREQ):
            pe_bs[i, i * NUM_FREQ + k] = 2.0 ** (k - 1)
            pe_bs[i, 24 + i * NUM_FREQ + k] = 2.0 ** (k - 1)
    pe_shift = np.zeros((48, 1), f32)
    pe_shift[24:48] = 0.25
    pe_bias = (pe_shift * f32(2 * np.pi)).astype(f32)

    # enc_W1 rows permuted to [sin/cos(48); x(4)], scaled to period units
    encW1 = inputs["enc_W1"].astype(f32)
    encW1p = np.concatenate([encW1[4:52], encW1[0:4]], axis=0) * SP

    d = {
        "pe_bs": pe_bs,
        "pe_shift": pe_shift,
        "pe_bias": pe_bias,
        "encW1a": np.ascontiguousarray(encW1p[0:48]).astype(f32),
        "encW1b": np.ascontiguousarray(encW1p[48:52]).astype(f32),
        "encW2r": (inputs["enc_W2"].astype(f32) * S30).astype(bf16),
        "polW0p": (inputs["pol_W0"].astype(f32)[0:IN_F] * SP),
        "polW1r": (inputs["pol_W1"].astype(f32) * S30),
        "polW2r": (inputs["pol_W2"].astype(f32) * S30),
        "polWl": inputs["pol_Wl"].astype(f32),
        "eW0a": np.ascontiguousarray(
            inputs["exp_W0"].astype(f32)[:, 0:ENC, :] * S30).astype(bf16),
        "eW0b": np.ascontiguousarray(
            inputs["exp_W0"].astype(f32)[:, ENC:ENC + POL, :] * S30).astype(bf16),
        "eW1": (inputs["exp_W1"].astype(f32) * S30).astype(bf16),
        "eW2": (inputs["exp_W2"].astype(f32) * S30).astype(bf16),
        "ident": np.eye(128, dtype=f32),
    }
    eWo = np.zeros((NE, EXP, NE), f32)
    for e in range(NE):
        eWo[e, :, e] = inputs["exp_Wo"][e, :, 0]
    d["eWo"] = eWo.astype(bf16)

    # biases are structurally zero in this model; the kernel folds none.
    for b in ["enc_b1", "enc_b2", "pol_b0", "pol_b1", "pol_b2", "pol_bl",
              "exp_b0", "exp_b1", "exp_b2", "exp_bo"]:
        assert not np.any(inputs[b]), f"nonzero bias {b} unsupported"

    return d


def _x_concat(x):
    """x [N,4] -> concatenated per-core xT blocks, shape (8*IN_F, NPC)."""
    f32 = np.float32
    return np.ascontiguousarray(
        x.astype(f32, copy=False).reshape(N_CORES, NPC, IN_F).transpose(0, 2, 1)
    ).reshape(N_CORES * IN_F, NPC)


_WKEYS = ["enc_W1", "enc_W2", "pol_W0", "pol_W1", "pol_W2", "pol_Wl",
          "exp_W0", "exp_W1", "exp_W2", "exp_Wo"]


def _weight_sig(inputs):
    return tuple(
        (k, id(inputs[k]), inputs[k].__array_interface__["data"][0])
        for k in _WKEYS
    )


def _make_fast(nc, wmap):
    """Cached jitted shard_map callable mirroring run_bass_via_pjrt exactly,
    with weights device-resident."""
    import jax
    from jax.sharding import Mesh, PartitionSpec, NamedSharding
    from jax.experimental.shard_map import shard_map
    from concourse import mybir
    from concourse.bass2jax import (
        _bass_exec_p,
        install_neuronx_cc_hook,
        partition_id_tensor,
    )

    install_neuronx_cc_hook()
    partition_name = nc.partition_id_tensor.name if nc.partition_id_tensor else None

    in_names, out_names, out_avals, zero_shapes = [], [], [], []
    for alloc in nc.m.functions[0].allocations:
        if not isinstance(alloc, mybir.MemoryLocationSet):
            continue
        name = alloc.memorylocations[0].name
        if alloc.kind == "ExternalInput":
            if name != partition_name:
                in_names.append(name)
        elif alloc.kind == "ExternalOutput":
            shape = tuple(alloc.tensor_shape)
            dtype = mybir.dt.np(alloc.dtype)
            out_avals.append(jax.core.ShapedArray(shape, dtype))
            zero_shapes.append((shape, dtype))
            out_names.append(name)
    n_params = len(in_names)
    n_outs = len(out_avals)
    in_names_full = in_names + out_names
    if partition_name is not None:
        in_names_full.append(partition_name)

    def _body(*args):
        operands = list(args)
        if partition_name is not None:
            operands.append(partition_id_tensor())
        outs = _bass_exec_p.bind(
            *operands,
            out_avals=tuple(out_avals),
            in_names=tuple(in_names_full),
            out_names=tuple(out_names),
            lowering_input_output_aliases=(),
            sim_require_finite=True,
            sim_require_nnan=True,
            nc=nc,
        )
        return tuple(outs)

    devices = jax.devices()[:N_CORES]
    mesh = Mesh(np.asarray(devices), ("core",))
    in_specs = (PartitionSpec("core"),) * (n_params + n_outs)
    out_specs = (PartitionSpec("core"),) * len(out_names)
    donate = tuple(range(n_params, n_params + n_outs))
    sharded = jax.jit(
        shard_map(_body, mesh=mesh, in_specs=in_specs, out_specs=out_specs,
                  check_rep=False),
        donate_argnums=donate,
        keep_unused=True,
    )
    shard = NamedSharding(mesh, PartitionSpec("core"))

    x_idx = in_names.index("xT")

    def put_weights(wmap):
        dev = {}
        for name in in_names:
            if name == "xT":
                continue
            w = wmap[name]
            rep = np.broadcast_to(w[None], (N_CORES, *w.shape)).reshape(
                N_CORES * w.shape[0], *w.shape[1:]
            )
            dev[name] = jax.device_put(np.ascontiguousarray(rep), shard)
        return dev

    state = {
        "sharded": sharded,
        "in_names": in_names,
        "out_names": out_names,
        "zero_shapes": zero_shapes,
        "x_idx": x_idx,
        "put_weights": put_weights,
        "dev_weights": put_weights(wmap),
    }
    return state


def _fast_call(state, x):
    args = []
    for name in state["in_names"]:
        if name == "xT":
            args.append(_x_concat(x))
        else:
            args.append(state["dev_weights"][name])
    zeros = [np.zeros((N_CORES * s[0], *s[1:]), dt)
             for (s, dt) in state["zero_shapes"]]
    out_arrs = state["sharded"](*args, *zeros)
    out = np.asarray(out_arrs[state["out_names"].index("out")])
    return out.reshape(N_TOTAL, 1).astype(np.float32)


def kernel(**inputs):
    from concourse.bass_utils import run_bass_kernel_spmd

    npc = NPC
    if npc not in _CACHE:
        _CACHE[npc] = _build(npc)
    nc = _CACHE[npc]

    sig = _weight_sig(inputs)
    st = _FAST.get(npc)
    if st is not None and st["sig"] == sig:
        return _fast_call(st["state"], inputs["x"])

    # First call (or weights changed): canonical run_bass_kernel_spmd path.
    wmap = _prep_weights(inputs)
    x = inputs["x"].astype(np.float32, copy=False)
    in_maps = []
    for core in range(N_CORES):
        m = dict(wmap)
        m["xT"] = np.ascontiguousarray(x[core * npc:(core + 1) * npc].T)
        in_maps.append(m)
    res = run_bass_kernel_spmd(nc, in_maps, list(range(N_CORES)))
    out = np.concatenate([res.results[c]["out"] for c in range(N_CORES)])
    out = out.reshape(N_TOTAL, 1).astype(np.float32)

    # Build/refresh the fast path for subsequent calls; validate it once.
    try:
        if st is None:
            st = {"state": _make_fast(nc, wmap), "sig": sig}
        else:
            st["state"]["dev_weights"] = st["state"]["put_weights"](wmap)
            st["sig"] = sig
        fast_out = _fast_call(st["state"], inputs["x"])
        if np.allclose(fast_out, out, rtol=1e-5, atol=1e-6):
            _FAST[npc] = st
        else:
            _FAST.pop(npc, None)
    except Exception:
        _FAST.pop(npc, None)

    return out


# revision 19
# speedup vs baseline: 1.0364x; 1.0296x over previous
"""ConditioningMoEINR Trainium2 kernel — dense 8-core data-parallel.

Device kernel: feature-major activations [feat, pts]; weights as stationary
lhsT.  Sin range reduction: magic-round pair (period units) for wide-range
layers (pe/enc1/pol0), single ADD_RANGE_WRAP (radian units) for bounded
layers.  Routing in point-major [128, (chunk,subtile)*7]; single Exp table
switch at the end; fast DVE reciprocal for the top-2 renormalization.

Host orchestration: the first call goes through run_bass_kernel_spmd (which
under axon lowers via bass2jax/PJRT and compiles+caches the NEFF).  That
path rebuilds the jitted shard_map closure and re-ships every replicated
weight on every call, so subsequent calls use a cached jitted callable with
device-resident weights — only x (2 MB) moves per call.
"""

import sys

if "/opt/trn_rl_repo" not in sys.path:
    sys.path.insert(0, "/opt/trn_rl_repo")

import numpy as np

# ---- problem constants (hardcoded per contract) ----
N_TOTAL = 131072
N_CORES = 8
NPC = N_TOTAL // N_CORES          # 16384 points per core
CHUNK = 512                       # matmul moving-dim tile
NCHUNK = NPC // CHUNK             # 32
GROUP = 2                         # chunks per expert psum group
NGROUP = NCHUNK // GROUP          # 16
NSUB = CHUNK // 128               # 4 point-subtiles per chunk
NJ = NCHUNK * NSUB                # 128 point-major column groups
NUM_FREQ = 6
IN_F = 4
ENC = 128
POL = 64
EXP = 128
NE = 7
OMEGA = 30.0

PI = float(np.pi)
TWO_PI = float(2 * np.pi)
MAGIC = float(np.float32(1.5 * 2 ** 23))
S_PER = float(np.float32(OMEGA / (2 * np.pi)))   # radians->period units
BIG = 1.0e30

_CACHE = {}
_FAST = {}


def _build(npc):
    import concourse.bacc as bacc
    import concourse.mybir as mybir
    import concourse.tile as tile
    from contextlib import ExitStack

    DT = mybir.dt.float32
    DTB = mybir.dt.bfloat16
    AF = mybir.ActivationFunctionType
    ALU = mybir.AluOpType

    nchunk = npc // CHUNK
    ngroup = nchunk // GROUP
    nj = nchunk * NSUB

    nc = bacc.Bacc("TRN2", target_bir_lowering=False, debug=False)

    def din(name, shape, dt=DT):
        return nc.dram_tensor(name, list(shape), dt, kind="ExternalInput").ap()

    xT = din("xT", (IN_F, npc))
    pe_bs = din("pe_bs", (IN_F, 48))          # period-unit freq matrix
    pe_shift = din("pe_shift", (48, 1))       # 0 / 0.25 (cos rows)
    pe_bias = din("pe_bias", (48, 1))         # 0 / pi/2
    encW1a = din("encW1a", (48, ENC))         # sin/cos rows * 30/2pi
    encW1b = din("encW1b", (IN_F, ENC))       # x rows * 30/2pi
    encW2r = din("encW2r", (ENC, ENC), DTB)   # * 30
    polW0p = din("polW0p", (IN_F, POL))       # * 30/2pi
    polW1r = din("polW1r", (POL, POL))        # * 30
    polW2r = din("polW2r", (POL, POL))        # * 30
    polWl = din("polWl", (POL, NE))
    eW0a = din("eW0a", (NE, ENC, EXP), DTB)   # * 30
    eW0b = din("eW0b", (NE, POL, EXP), DTB)   # * 30
    eW1 = din("eW1", (NE, EXP, EXP), DTB)     # * 30
    eW2 = din("eW2", (NE, EXP, EXP), DTB)     # * 30
    eWo = din("eWo", (NE, EXP, NE), DTB)      # col-e padded Wo
    ident = din("ident", (128, 128))
    out_d = nc.dram_tensor("out", [npc], DT, kind="ExternalOutput").ap()

    with tile.TileContext(nc) as tc, ExitStack() as ctx:
        wpool = ctx.enter_context(tc.tile_pool(name="w", bufs=1))
        spool = ctx.enter_context(tc.tile_pool(name="s", bufs=3))
        s2pool = ctx.enter_context(tc.tile_pool(name="s2", bufs=4))
        gpool = ctx.enter_context(tc.tile_pool(name="g", bufs=3))
        rpool = ctx.enter_context(tc.tile_pool(name="r", bufs=1))
        ppool = ctx.enter_context(tc.tile_pool(name="ps", bufs=2, space="PSUM"))
        tpool = ctx.enter_context(tc.tile_pool(name="tr", bufs=1, space="PSUM"))

        # ---- resident weights ----
        _wn = [0]

        def wload(ap, shape, dt=DT):
            _wn[0] += 1
            t = wpool.tile(list(shape), dt, name=f"w{_wn[0]}", tag=f"w{_wn[0]}")
            nc.sync.dma_start(t[:], ap)
            return t

        w_pebs = wload(pe_bs[:], (IN_F, 48))
        w_peshift = wload(pe_shift[:], (48, 1))
        w_pebias = wload(pe_bias[:], (48, 1))
        w_enc1a = wload(encW1a[:], (48, ENC))
        w_enc1b = wload(encW1b[:], (IN_F, ENC))
        w_enc2 = wload(encW2r[:], (ENC, ENC), DTB)
        w_pol0 = wload(polW0p[:], (IN_F, POL))
        w_pol1 = wload(polW1r[:], (POL, POL))
        w_pol2 = wload(polW2r[:], (POL, POL))
        w_polWl = wload(polWl[:], (POL, NE))
        w_e0a = [wload(eW0a[e], (ENC, EXP), DTB) for e in range(NE)]
        w_e0b = [wload(eW0b[e], (POL, EXP), DTB) for e in range(NE)]
        w_e1 = [wload(eW1[e], (EXP, EXP), DTB) for e in range(NE)]
        w_e2 = [wload(eW2[e], (EXP, EXP), DTB) for e in range(NE)]
        w_eo = [wload(eWo[e], (EXP, NE), DTB) for e in range(NE)]
        w_id = wload(ident[:], (128, 128))

        # point-major logits / preds for the whole core
        LT = rpool.tile([128, nj * NE], DT)
        PT = rpool.tile([128, nj * NE], DT)

        def magic_sin(dst, psum_ap, p, n, shift_ap, bias_ap):
            """dst = Sin(-2pi*((psum+shift+M)-M-psum) + bias); psum in period units."""
            tsh = spool.tile([p, n], DT, tag="tsh")
            if shift_ap is None:
                nc.vector.tensor_scalar_add(tsh[:p, :n], psum_ap, MAGIC)
            else:
                nc.vector.tensor_scalar(
                    tsh[:p, :n], psum_ap, shift_ap, MAGIC, op0=ALU.add, op1=ALU.add
                )
            u = spool.tile([p, n], DT, tag="u")
            nc.vector.scalar_tensor_tensor(
                u[:p, :n], tsh[:p, :n], MAGIC, psum_ap,
                op0=ALU.subtract, op1=ALU.subtract,
            )
            if bias_ap is None:
                nc.scalar.activation(dst, u[:p, :n], AF.Sin, scale=-TWO_PI)
            else:
                nc.scalar.activation(dst, u[:p, :n], AF.Sin, bias=bias_ap, scale=-TWO_PI)

        def wrap_sin(dst, psum_ap, p, n):
            """dst = Sin(wrap(psum)); psum in radians, |arg| < 3pi.

            The wrap lands in SBUF so the PSUM bank frees after the DVE
            read and the Sin runs off the PE critical path."""
            w = gpool.tile([p, n], DT, tag=f"wr{p}_{n}")
            nc.vector.add_range_wrap(w[:p, :n], psum_ap, shift=0.0, bound=PI, period=TWO_PI)
            nc.scalar.activation(dst, w[:p, :n], AF.Sin)

        # HAM warmup: back-to-back dummy matmuls during the weight-DMA wait
        # flip the PE clock gate to 8/8 before real work starts.
        warm = ppool.tile([128, CHUNK], DT, tag="trunk")
        for _ in range(10):
            nc.tensor.matmul(warm[0:48, 0:128], w_pebs[:], w_enc1b[:],
                             start=True, stop=True)

        for g in range(ngroup):
            xg = spool.tile([IN_F, GROUP * CHUNK], DT, tag="xg")
            nc.sync.dma_start(xg[:], xT[:, g * GROUP * CHUNK:(g + 1) * GROUP * CHUNK])
            s2cat = s2pool.tile([ENC, GROUP * CHUNK], DTB, tag="s2cat")
            sp2cat = s2pool.tile([POL, GROUP * CHUNK], DTB, tag="sp2cat")
            for ci in range(GROUP):
                c = g * GROUP + ci
                xs = xg[:, ci * CHUNK:(ci + 1) * CHUNK]

                # --- positional encoding ---
                t48 = ppool.tile([128, CHUNK], DT, tag="trunk")
                nc.tensor.matmul(t48[0:48, :], w_pebs[:], xs, start=True, stop=True)
                pesin = spool.tile([48, CHUNK], DT, tag="pesin")
                magic_sin(pesin[:], t48[0:48, :], 48, CHUNK, w_peshift[:], w_pebias[:])

                # --- encoder ---
                h1 = ppool.tile([128, CHUNK], DT, tag="trunk")
                nc.tensor.matmul(h1[:], w_enc1a[:], pesin[:], start=True, stop=False)
                nc.tensor.matmul(h1[:], w_enc1b[:], xs, start=False, stop=True)
                s1 = spool.tile([ENC, CHUNK], DTB, tag="s1")
                magic_sin(s1[:], h1[:], ENC, CHUNK, None, None)

                h2 = ppool.tile([128, CHUNK], DT, tag="trunk")
                nc.tensor.matmul(h2[:], w_enc2[:], s1[:], start=True, stop=True)
                wrap_sin(s2cat[:, ci * CHUNK:(ci + 1) * CHUNK], h2[:], ENC, CHUNK)

                # --- policy ---
                p0 = ppool.tile([128, CHUNK], DT, tag="trunk")
                nc.tensor.matmul(p0[0:POL, :], w_pol0[:], xs, start=True, stop=True)
                sp0 = spool.tile([POL, CHUNK], DT, tag="sp0")
                magic_sin(sp0[:], p0[0:POL, :], POL, CHUNK, None, None)

                p1 = ppool.tile([128, CHUNK], DT, tag="trunk")
                nc.tensor.matmul(p1[0:POL, :], w_pol1[:], sp0[:], start=True, stop=True)
                sp1 = spool.tile([POL, CHUNK], DT, tag="sp1")
                wrap_sin(sp1[:], p1[0:POL, :], POL, CHUNK)

                p2 = ppool.tile([128, CHUNK], DT, tag="trunk")
                nc.tensor.matmul(p2[0:POL, :], w_pol2[:], sp1[:], start=True, stop=True)
                sp2 = s2pool.tile([POL, CHUNK], DT, tag="sp2")
                wrap_sin(sp2[:], p2[0:POL, :], POL, CHUNK)
                nc.vector.tensor_copy(sp2cat[:, ci * CHUNK:(ci + 1) * CHUNK], sp2[:])

                # --- logits, point-major [128, 28] ---
                lt = ppool.tile([128, NSUB * NE], DT, tag="trunk")
                for s in range(NSUB):
                    nc.tensor.matmul(
                        lt[:, s * NE:(s + 1) * NE],
                        sp2[:, s * 128:(s + 1) * 128],
                        w_polWl[:],
                        start=True, stop=True,
                    )
                nc.vector.tensor_copy(LT[:, c * NSUB * NE:(c + 1) * NSUB * NE], lt[:, 0:NSUB * NE])

            # --- experts (bf16, 512-col matmuls: ISA caps MM out at 512 f32) ---
            p7 = tpool.tile([NE, GROUP * CHUNK], DT, tag="p7", name=f"p7_{g}")
            GC = GROUP * CHUNK

            def halves(psum, w, src):
                for ci in range(GROUP):
                    sl = slice(ci * CHUNK, (ci + 1) * CHUNK)
                    nc.tensor.matmul(psum[:, sl], w[:], src[:, sl], start=True, stop=True)

            for e in range(NE):
                x0 = ppool.tile([128, GC], DT, tag="exp")
                for ci in range(GROUP):
                    sl = slice(ci * CHUNK, (ci + 1) * CHUNK)
                    nc.tensor.matmul(x0[:, sl], w_e0a[e][:], s2cat[:, sl], start=True, stop=False)
                    nc.tensor.matmul(x0[:, sl], w_e0b[e][:], sp2cat[:, sl], start=False, stop=True)
                g0 = gpool.tile([EXP, GC], DTB, tag="g0")
                wrap_sin(g0[:], x0[:], EXP, GC)

                x1 = ppool.tile([128, GC], DT, tag="exp")
                halves(x1, w_e1[e], g0)
                g1 = gpool.tile([EXP, GC], DTB, tag="g1")
                wrap_sin(g1[:], x1[:], EXP, GC)

                x2 = ppool.tile([128, GC], DT, tag="exp")
                halves(x2, w_e2[e], g1)
                g2 = gpool.tile([EXP, GC], DTB, tag="g2")
                wrap_sin(g2[:], x2[:], EXP, GC)

                for ci in range(GROUP):
                    sl = slice(ci * CHUNK, (ci + 1) * CHUNK)
                    nc.tensor.matmul(
                        p7[0:NE, sl], w_eo[e][:], g2[:, sl],
                        start=(e == 0), stop=(e == NE - 1),
                    )

            # preds -> point-major PT via PE transpose
            tmp7 = spool.tile([NE, GROUP * CHUNK], DT, tag="tmp7")
            nc.vector.tensor_copy(tmp7[:], p7[0:NE, :])
            for ci in range(GROUP):
                c = g * GROUP + ci
                tp = ppool.tile([128, NSUB * NE], DT, tag="trunk")
                for s in range(NSUB):
                    nc.tensor.transpose(
                        tp[:, s * NE:(s + 1) * NE],
                        tmp7[:, ci * CHUNK + s * 128:ci * CHUNK + (s + 1) * 128],
                        w_id[0:NE, 0:NE],
                    )
                nc.vector.tensor_copy(PT[:, c * NSUB * NE:(c + 1) * NSUB * NE], tp[:, 0:NSUB * NE])

        # ================= phase B: routing + combine =================
        LT3 = LT[:].rearrange("p (j e) -> p j e", e=NE)

        def etree(op, src3, width_tag):
            """pairwise tree over the 7-expert innermost dim -> [128, nj]."""
            m4 = rpool.tile([128, nj * 4], DT, tag=f"{width_tag}4")
            m43 = m4[:].rearrange("p (j e) -> p j e", e=4)
            nc.vector.tensor_tensor(m43, src3[:, :, 0:4], src3[:, :, 3:7], op)
            m2 = rpool.tile([128, nj * 2], DT, tag=f"{width_tag}2")
            m23 = m2[:].rearrange("p (j e) -> p j e", e=2)
            nc.vector.tensor_tensor(m23, m43[:, :, 0:2], m43[:, :, 2:4], op)
            m1 = rpool.tile([128, nj], DT, tag=f"{width_tag}1")
            m13 = m1[:].rearrange("p (j e) -> p j e", e=1)
            nc.vector.tensor_tensor(m13, m23[:, :, 0:1], m23[:, :, 1:2], op)
            return m1

        def erep(m1, tag):
            """broadcast [128, nj] -> [128, nj*7] along innermost expert dim."""
            r = rpool.tile([128, nj * NE], DT, tag=tag)
            r3 = r[:].rearrange("p (j e) -> p j e", e=NE)
            m13 = m1[:].rearrange("p (j e) -> p j e", e=1)
            nc.vector.tensor_copy(r3[:, :, 0:1], m13)
            nc.vector.tensor_copy(r3[:, :, 1:2], r3[:, :, 0:1])
            nc.vector.tensor_copy(r3[:, :, 2:4], r3[:, :, 0:2])
            nc.vector.tensor_copy(r3[:, :, 4:7], r3[:, :, 1:4])
            return r, r3

        mx1 = etree(ALU.max, LT3, "mxa")
        rep1, rep13 = erep(mx1, "rep1")
        # masked' = BIG*ge1 - LT  (negated; use min-tree then negate)
        ge1 = rpool.tile([128, nj * NE], DT, tag="ge1")
        nc.vector.tensor_tensor(ge1[:], LT[:], rep1[:], ALU.is_ge)
        maskd = rpool.tile([128, nj * NE], DT, tag="maskd")
        nc.vector.scalar_tensor_tensor(
            maskd[:], ge1[:], BIG, LT[:], op0=ALU.mult, op1=ALU.subtract
        )
        mn = etree(ALU.min, maskd[:].rearrange("p (j e) -> p j e", e=NE), "mna")
        mx2 = rpool.tile([128, nj], DT, tag="mx2")
        nc.vector.tensor_scalar_mul(mx2[:], mn[:], -1.0)
        rep2, _ = erep(mx2, "rep2")
        keep = rpool.tile([128, nj * NE], DT, tag="keep")
        nc.vector.tensor_tensor(keep[:], LT[:], rep2[:], ALU.is_ge)

        ex = rpool.tile([128, nj * NE], DT, tag="ex")
        nc.scalar.activation(ex[:], LT[:], AF.Exp)
        ew = rpool.tile([128, nj * NE], DT, tag="ew")
        nc.vector.tensor_tensor(ew[:], ex[:], keep[:], ALU.mult)
        wp = rpool.tile([128, nj * NE], DT, tag="wp")
        nc.vector.tensor_tensor(wp[:], ew[:], PT[:], ALU.mult)

        den = rpool.tile([128, nj], DT, tag="den")
        nc.vector.tensor_reduce(
            den[:], ew[:].rearrange("p (j e) -> p j e", e=NE),
            mybir.AxisListType.X, ALU.add,
        )
        num = rpool.tile([128, nj], DT, tag="num")
        nc.vector.tensor_reduce(
            num[:], wp[:].rearrange("p (j e) -> p j e", e=NE),
            mybir.AxisListType.X, ALU.add,
        )
        rec = rpool.tile([128, nj], DT, tag="rec")
        scratch = rpool.tile([128, nj], DT, tag="recs")
        nc.vector.reciprocal_approx_accurate(rec[:], den[:], scratch[:])
        outv = rpool.tile([128, nj], DT, tag="outv")
        nc.vector.tensor_tensor(outv[:], num[:], rec[:], ALU.mult)

        # transpose [128 q, nj] -> [nj, 128 q] and store
        for b in range(nj // 128):
            tp = ppool.tile([128, 128], DT, tag="exp")
            nc.tensor.transpose(tp[:, 0:128], outv[:, b * 128:(b + 1) * 128], w_id[:])
            osb = rpool.tile([128, 128], DT, tag="osb")
            nc.vector.tensor_copy(osb[:], tp[:, 0:128])
            nc.sync.dma_start(
                out_d.rearrange("(j q) -> j q", q=128)[b * 128:(b + 1) * 128, :],
                osb[:],
            )

    nc.compile()
    return nc


def _prep_weights(inputs):
    """Weight-derived device inputs (shared by all cores)."""
    import ml_dtypes

    f32 = np.float32
    bf16 = ml_dtypes.bfloat16
    S30 = f32(OMEGA)
    SP = f32(OMEGA / (2 * np.pi))

    # pe freq matrix in period units: col j=i*6+k (sin), 24+j (cos) = 2^(k-1)
    pe_bs = np.zeros((IN_F, 48), f32)
    for i in range(IN_F):
        for k in range(NUM_FREQ):
            pe_bs[i, i * NUM_FREQ + k] = 2.0 ** (k - 1)
            pe_bs[i, 24 + i * NUM_FREQ + k] = 2.0 ** (k - 1)
    pe_shift = np.zeros((48, 1), f32)
    pe_shift[24:48] = 0.25
    pe_bias = (pe_shift * f32(2 * np.pi)).astype(f32)

    # enc_W1 rows permuted to [sin/cos(48); x(4)], scaled to period units
    encW1 = inputs["enc_W1"].astype(f32)
    encW1p = np.concatenate([encW1[4:52], encW1[0:4]], axis=0) * SP

    d = {
        "pe_bs": pe_bs,
        "pe_shift": pe_shift,
        "pe_bias": pe_bias,
        "encW1a": np.ascontiguousarray(encW1p[0:48]).astype(f32),
        "encW1b": np.ascontiguousarray(encW1p[48:52]).astype(f32),
        "encW2r": (inputs["enc_W2"].astype(f32) * S30).astype(bf16),
        "polW0p": (inputs["pol_W0"].astype(f32)[0:IN_F] * SP),
        "polW1r": (inputs["pol_W1"].astype(f32) * S30),
        "polW2r": (inputs["pol_W2"].astype(f32) * S30),
        "polWl": inputs["pol_Wl"].astype(f32),
        "eW0a": np.ascontiguousarray(
            inputs["exp_W0"].astype(f32)[:, 0:ENC, :] * S30).astype(bf16),
        "eW0b": np.ascontiguousarray(
            inputs["exp_W0"].astype(f32)[:, ENC:ENC + POL, :] * S30).astype(bf16),
        "eW1": (inputs["exp_W1"].astype(f32) * S30).astype(bf16),
        "eW2": (inputs["exp_W2"].astype(f32) * S30).astype(bf16),
        "ident": np.eye(128, dtype=f32),
    }
    eWo = np.zeros((NE, EXP, NE), f32)
    for e in range(NE):
        eWo[e, :, e] = inputs["exp_Wo"][e, :, 0]
    d["eWo"] = eWo.astype(bf16)

    # biases are structurally zero in this model; the kernel folds none.
    for b in ["enc_b1", "enc_b2", "pol_b0", "pol_b1", "pol_b2", "pol_bl",
              "exp_b0", "exp_b1", "exp_b2", "exp_bo"]:
        assert not np.any(inputs[b]), f"nonzero bias {b} unsupported"

    return d


def _x_concat(x):
    """x [N,4] -> concatenated per-core xT blocks, shape (8*IN_F, NPC)."""
    f32 = np.float32
    return np.ascontiguousarray(
        x.astype(f32, copy=False).reshape(N_CORES, NPC, IN_F).transpose(0, 2, 1)
    ).reshape(N_CORES * IN_F, NPC)


_WKEYS = ["enc_W1", "enc_W2", "pol_W0", "pol_W1", "pol_W2", "pol_Wl",
          "exp_W0", "exp_W1", "exp_W2", "exp_Wo"]


def _weight_sig(inputs):
    return tuple(
        (k, id(inputs[k]), inputs[k].__array_interface__["data"][0])
        for k in _WKEYS
    )


def _make_fast(nc, wmap):
    """Cached jitted shard_map callable mirroring run_bass_via_pjrt exactly,
    with weights device-resident."""
    import jax
    from jax.sharding import Mesh, PartitionSpec, NamedSharding
    from jax.experimental.shard_map import shard_map
    from concourse import mybir
    from concourse.bass2jax import (
        _bass_exec_p,
        install_neuronx_cc_hook,
        partition_id_tensor,
    )

    install_neuronx_cc_hook()
    partition_name = nc.partition_id_tensor.name if nc.partition_id_tensor else None

    in_names, out_names, out_avals, zero_shapes = [], [], [], []
    for alloc in nc.m.functions[0].allocations:
        if not isinstance(alloc, mybir.MemoryLocationSet):
            continue
        name = alloc.memorylocations[0].name
        if alloc.kind == "ExternalInput":
            if name != partition_name:
                in_names.append(name)
        elif alloc.kind == "ExternalOutput":
            shape = tuple(alloc.tensor_shape)
            dtype = mybir.dt.np(alloc.dtype)
            out_avals.append(jax.core.ShapedArray(shape, dtype))
            zero_shapes.append((shape, dtype))
            out_names.append(name)
    n_params = len(in_names)
    n_outs = len(out_avals)
    in_names_full = in_names + out_names
    if partition_name is not None:
        in_names_full.append(partition_name)

    def _body(*args):
        operands = list(args)
        if partition_name is not None:
            operands.append(partition_id_tensor())
        outs = _bass_exec_p.bind(
            *operands,
            out_avals=tuple(out_avals),
            in_names=tuple(in_names_full),
            out_names=tuple(out_names),
            lowering_input_output_aliases=(),
            sim_require_finite=True,
            sim_require_nnan=True,
            nc=nc,
        )
        return tuple(outs)

    devices = jax.devices()[:N_CORES]
    mesh = Mesh(np.asarray(devices), ("core",))
    in_specs = (PartitionSpec("core"),) * (n_params + n_outs)
    out_specs = (PartitionSpec("core"),) * len(out_names)
    donate = tuple(range(n_params, n_params + n_outs))
    sharded = jax.jit(
        shard_map(_body, mesh=mesh, in_specs=in_specs, out_specs=out_specs,
                  check_rep=False),
        donate_argnums=donate,
        keep_unused=True,
    )
    shard = NamedSharding(mesh, PartitionSpec("core"))

    x_idx = in_names.index("xT")

    def put_weights(wmap):
        dev = {}
        for name in in_names:
            if name == "xT":
                continue
            w = wmap[name]
            rep = np.broadcast_to(w[None], (N_CORES, *w.shape)).reshape(
                N_CORES * w.shape[0], *w.shape[1:]
            )
            dev[name] = jax.device_put(np.ascontiguousarray(rep), shard)
        return dev

    state = {
        "sharded": sharded,
        "in_names": in_names,
        "out_names": out_names,
        "zero_shapes": zero_shapes,
        "x_idx": x_idx,
        "put_weights": put_weights,
        "dev_weights": put_weights(wmap),
    }
    return state


def _fast_call(state, x):
    args = []
    for name in state["in_names"]:
        if name == "xT":
            args.append(_x_concat(x))
        else:
            args.append(state["dev_weights"][name])
    zeros = [np.zeros((N_CORES * s[0], *s[1:]), dt)
             for (s, dt) in state["zero_shapes"]]
    out_arrs = state["sharded"](*args, *zeros)
    out = np.asarray(out_arrs[state["out_names"].index("out")])
    return out.reshape(N_TOTAL, 1).astype(np.float32)


def kernel(**inputs):
    from concourse.bass_utils import run_bass_kernel_spmd

    npc = NPC
    if npc not in _CACHE:
        _CACHE[npc] = _build(npc)
    nc = _CACHE[npc]

    sig = _weight_sig(inputs)
    st = _FAST.get(npc)
    if st is not None and st["sig"] == sig:
        return _fast_call(st["state"], inputs["x"])

    # First call (or weights changed): canonical run_bass_kernel_spmd path.
    wmap = _prep_weights(inputs)
    x = inputs["x"].astype(np.float32, copy=False)
    in_maps = []
    for core in range(N_CORES):
        m = dict(wmap)
        m["xT"] = np.ascontiguousarray(x[core * npc:(core + 1) * npc].T)
        in_maps.append(m)
    res = run_bass_kernel_spmd(nc, in_maps, list(range(N_CORES)))
    out = np.concatenate([res.results[c]["out"] for c in range(N_CORES)])
    out = out.reshape(N_TOTAL, 1).astype(np.float32)

    # Build/refresh the fast path for subsequent calls; validate it once.
    try:
        if st is None:
            st = {"state": _make_fast(nc, wmap), "sig": sig}
        else:
            st["state"]["dev_weights"] = st["state"]["put_weights"](wmap)
            st["sig"] = sig
        fast_out = _fast_call(st["state"], inputs["x"])
        if np.allclose(fast_out, out, rtol=1e-5, atol=1e-6):
            _FAST[npc] = st
        else:
            _FAST.pop(npc, None)
    except Exception:
        _FAST.pop(npc, None)

    return out
